# revision 62
# baseline (speedup 1.0000x reference)
"""BERT self-attention (no mask) on 8 TRN2 NeuronCores, head-parallel.

Full inputs in, full output out. Core c computes heads 2c and 2c+1 (output
hidden columns [c*128, (c+1)*128)). All matmuls run in bf16 (1 cycle/row at
any width). X^T is produced by DMA xbar transposes of a Pool-engine bf16
copy of X, so the PE array does no transposes at all. Attention scores are
computed transposed (scores^T[k, q]); softmax denominators come from a
ones-column appended to the natural-layout V tiles, and the PV matmul is
oriented with probs as the stationary operand so it streams only 65
columns per key tile. Exp runs on ACT in 1536-wide tiles to amortize the
SBUF access overhead; projection of batch b+1 is interleaved with
attention of batch b.
"""

import numpy as np

try:
    import concourse.bass as bass
except ImportError:  # toolchain not on sys.path in the caller's environment
    import sys
    sys.path.insert(0, "/opt/trn_rl_repo")
    import concourse.bass as bass
import concourse.bacc as bacc
import concourse.mybir as mybir
import concourse.tile as tile
from concourse.bass_utils import run_bass_kernel_spmd

F32 = mybir.dt.float32
BF16 = mybir.dt.bfloat16

B = 4
S = 2048
H = 1024
NH = 16
HD = 64
NSEQ = B * S  # 8192
NCORES = 8
CSLICE = H // NCORES  # 128 hidden cols per core = 2 heads
KCH = H // 128  # 8 contraction tiles for projections
ST = S // 128  # 16 seq tiles of 128 rows per batch
KT = S // 128  # 16 key tiles per (b, head)
QC = 4  # q-chunks of 512 per (b, head)
QW = S // QC  # 512
VW = HD + 1  # V tile width incl. ones column
GW = 3 * 512  # exp group width (3 score blocks)

_STATE = None


def _build():
    nc = bacc.Bacc("TRN2", target_bir_lowering=False, debug=False,
                   num_devices=NCORES)

    x = nc.dram_tensor("x", [NSEQ, H], F32, kind="ExternalInput").ap()
    ws = {n: nc.dram_tensor(f"w{n}", [H, CSLICE], F32, kind="ExternalInput").ap()
          for n in "qkv"}
    bs = {n: nc.dram_tensor(f"b{n}", [CSLICE, 1], F32, kind="ExternalInput").ap()
          for n in "qkv"}
    out = nc.dram_tensor("out", [NSEQ, CSLICE], F32, kind="ExternalOutput").ap()

    with tile.TileContext(nc) as tc:
        with (
            tc.tile_pool(name="persist", bufs=1) as persist,
            tc.tile_pool(name="wstg", bufs=4) as wstg_pool,
            tc.tile_pool(name="xn", bufs=4) as xn_pool,
            tc.tile_pool(name="xnb", bufs=6) as xnb_pool,
            tc.tile_pool(name="xt", bufs=2) as xt_pool,
            tc.tile_pool(name="qkvt", bufs=2) as qkvt_pool,
            tc.tile_pool(name="vp", bufs=4) as vp_pool,
            tc.tile_pool(name="pr", bufs=6) as pr_pool,
            tc.tile_pool(name="rc", bufs=8) as rc_pool,
            tc.tile_pool(name="ot", bufs=4) as ot_pool,
            tc.tile_pool(name="spsum", bufs=2, space="PSUM") as spsum,
            tc.tile_pool(name="cpsum", bufs=1, space="PSUM") as cpsum,
            tc.tile_pool(name="ppsum", bufs=1, space="PSUM") as ppsum,
        ):
            # weights fp32 -> bf16, staged through the ACT DMA queue
            wt = {}
            for n in "qkv":
                for kk in range(KCH):
                    stg = wstg_pool.tile([128, CSLICE], F32,
                                         tag="wstage", name="wstage")
                    nc.scalar.dma_start(stg, ws[n][kk * 128:(kk + 1) * 128, :])
                    t = persist.tile([128, CSLICE], BF16,
                                     tag=f"w{n}{kk}", name=f"w{n}{kk}")
                    nc.vector.tensor_copy(t, stg)
                    wt[n, kk] = t
            bt = {}
            for n in "qkv":
                t = persist.tile([128, 1], F32, tag=f"b{n}", name=f"b{n}")
                nc.scalar.dma_start(t, bs[n])
                bt[n] = t

            def load_cast(b, st0, xnbs):
                # prologue only: SWDGE casting DMA lands bf16 directly,
                # halving descriptor cost and skipping the Pool convert
                for st in range(st0, st0 + 4):
                    r0 = b * S + st * 128
                    xnb = xnb_pool.tile([128, H], BF16, tag="xnb",
                                        name="xnb")
                    nc.gpsimd.dma_start(xnb, x[r0:r0 + 128, :])
                    xnbs[st] = xnb

            def load_quarter(b, st0, xnbs):
                # pair-loads: 2 seq-tiles per DMA halve the issue overhead
                # without a long exclusive hold on the DMA engines
                for h in range(2):
                    r0 = b * S + (st0 + 2 * h) * 128
                    xn = xn_pool.tile([128, 2, H], F32, tag="xn", name="xn")
                    nc.sync.dma_start(
                        xn, x[r0:r0 + 256, :].rearrange(
                            "(j p) c -> p j c", p=128))
                    for j in range(2):
                        xnb = xnb_pool.tile([128, H], BF16, tag="xnb",
                                            name="xnb")
                        nc.gpsimd.tensor_copy(xnb, xn[:, j, :])
                        xnbs[st0 + 2 * h + j] = xnb

            def xt_st(b, st, xnbs, xts):
                # XT[c, st, kk, r] = xnb[r, kk*128 + c]
                nc.sync.dma_start_transpose(xts[:, st, :, :], xnbs.pop(st))

            def proj_mm(n, ci, kk, xts, pps):
                if kk == 0:
                    pps[n, ci] = ppsum.tile([128, QW], F32, tag="pp",
                                            name=f"pp{n}")
                nc.tensor.matmul(
                    pps[n, ci], wt[n, kk], xts[:, 4 * ci:4 * ci + 4, kk, :],
                    start=(kk == 0), stop=(kk == KCH - 1))

            def proj_bias(n, ci, qkvT, pps):
                nc.vector.tensor_scalar_add(
                    qkvT[n][:, ci * QW:(ci + 1) * QW], pps.pop((n, ci)),
                    bt[n])

            def prep_v(b, hl, qkvT, vps):
                # natural-layout V tiles + ones column for the denominator.
                # xbar transpose needs a contiguous destination, so land in
                # tmp and interleave the ones column with a DVE copy.
                vp = vps[hl]
                tmp = xnb_pool.tile([128, KT, HD], BF16, tag="vtmp",
                                    name="vtmp")
                nc.sync.dma_start_transpose(
                    tmp, qkvT["v"][hl * HD:(hl + 1) * HD, :])
                nc.vector.memset(vp[:, :, HD:VW], 1.0)
                nc.vector.tensor_copy(vp[:, :, 0:HD], tmp)

            def score_step(b, blocks, qkvT, prs):
                # scores for a group of blocks (hl, qc, kt), then exp
                gw = len(blocks) * 512
                s_ps = spsum.tile([128, GW], F32, tag="s", name="s")
                for bi, (hl, qc, kt) in enumerate(blocks):
                    p0 = hl * HD
                    nc.tensor.matmul(
                        s_ps[:, bi * 512:(bi + 1) * 512],
                        qkvT["k"][p0:p0 + HD, kt * 128:(kt + 1) * 128],
                        qkvT["q"][p0:p0 + HD, qc * QW:(qc + 1) * QW],
                        start=True, stop=True)
                pr = pr_pool.tile([128, GW], BF16, tag="pr", name="pr")
                nc.scalar.activation(
                    pr[:, 0:gw], s_ps[:, 0:gw],
                    mybir.ActivationFunctionType.Exp,
                    scale=1.0 / np.sqrt(float(HD)))
                prs.append((blocks, pr))

            def pv_step(b, qkvT, vps, ctxs, prs):
                blocks, pr = prs.pop(0)
                for bi, (hl, qc, kt) in enumerate(blocks):
                    if kt == 0:
                        ctxs[hl, qc] = cpsum.tile([128, 4 * VW], F32,
                                                  tag="ctx", name="ctx")
                    ctx = ctxs[hl, qc]
                    # start=True arms the whole PSUM bank (first write per
                    # address replaces, rest accumulate), so arm exactly once
                    # per ctx tile or later slices wipe earlier ones' kt=0.
                    for qs in range(4):
                        nc.tensor.matmul(
                            ctx[:, qs * VW:(qs + 1) * VW],
                            pr[:, bi * 512 + qs * 128:bi * 512 + (qs + 1) * 128],
                            vps[hl][:, kt, :],
                            start=(kt == 0 and qs == 0), stop=(kt == KT - 1),
                            skip_group_check=True)
                    if kt == KT - 1:
                        finish_qc(b, hl, qc, ctxs)

            def finish_qc(b, hl, qc, ctxs):
                ctx = ctxs.pop((hl, qc))
                rc = rc_pool.tile([128, 4], F32, tag="rc", name="rc")
                nc.vector.reciprocal(rc, ctx[:, HD::VW])
                ot = ot_pool.tile([128, 4, HD], F32, tag="ot", name="ot")
                for qs in range(4):
                    nc.vector.tensor_scalar_mul(
                        ot[:, qs, :], ctx[:, qs * VW:qs * VW + HD],
                        rc[:, qs:qs + 1])
                r0 = b * S + qc * QW
                dst = out[r0:r0 + QW, hl * HD:(hl + 1) * HD].rearrange(
                    "(q p) c -> p q c", p=128)
                nc.sync.dma_start(dst, ot)

            def alloc_batch(b):
                xts = xt_pool.tile([128, ST, KCH, 128], BF16,
                                   tag="xt", name="xt")
                qkvT = {n: qkvt_pool.tile([128, S], BF16,
                                          tag=f"{n}T", name=f"{n}T")
                        for n in "qkv"}
                vps = [vp_pool.tile([128, KT, VW], BF16,
                                    tag=f"vp{hl}", name=f"vp{hl}")
                       for hl in range(2)]
                return xts, qkvT, vps

            def proj_steps(b, xts, qkvT, vps, prologue=False):
                # prologue=True interleaves loads with the transposes so the
                # first X^T tiles don't queue behind 24us of load transfers
                # on the DMA engines; in an attention window the loads go
                # up front so they never contend with the previous batch's.
                pps = {}
                xnbs = {}
                steps = []
                if not prologue:
                    for st0 in range(0, ST, 4):
                        steps.append(lambda st0=st0: load_quarter(
                            b, st0, xnbs))
                else:
                    steps.append(lambda: load_cast(b, 0, xnbs))
                for ci in range(QC):
                    if prologue and ci + 1 < QC:
                        steps.append(lambda ci=ci: load_cast(
                            b, 4 * ci + 4, xnbs))
                    steps.append(lambda ci=ci: [
                        xt_st(b, st, xnbs, xts)
                        for st in range(4 * ci, 4 * ci + 4)])
                    for n in "qkv":
                        for kk in range(KCH):
                            steps.append(lambda n=n, ci=ci, kk=kk: proj_mm(
                                n, ci, kk, xts, pps))
                        steps.append(lambda n=n, ci=ci: proj_bias(
                            n, ci, qkvT, pps))
                for hl in range(2):
                    steps.append(lambda hl=hl: prep_v(b, hl, qkvT, vps))
                return steps, None

            def att_lists(b, qkvT, vps):
                # separate score-step and pv-step lists so the driver can
                # run the exp stream ahead of PV (which needs vp ready)
                blocks = [(hl, qc, kt)
                          for hl in range(2) for qc in range(QC)
                          for kt in range(KT)]
                groups = [blocks[i:i + 3] for i in range(0, len(blocks), 3)]
                ctxs = {}
                prs = []
                ss = [lambda g=g: score_step(b, g, qkvT, prs)
                      for g in groups]
                pv = [lambda: pv_step(b, qkvT, vps, ctxs, prs)
                      for _ in groups]
                return ss, pv

            # one continuous scores/PV conveyor across all batches: strict
            # ss(i)/pv(i-1) alternation crosses batch joins so there is no
            # pipeline drain at batch boundaries; projection of batch b+1
            # is paced across batch b's stretch of the stream, finishing
            # ~85% in.
            state = {}
            state[0] = alloc_batch(0)
            for b in range(B):
                ss, pv = att_lists(b, state[b][1], state[b][2])
                if b == 0:
                    for step in proj_steps(0, *state[0],
                                           prologue=True)[0]:
                        step()
                nxt = []
                if b + 1 < B:
                    state[b + 1] = alloc_batch(b + 1)
                    nxt, _ = proj_steps(b + 1, *state[b + 1])
                att = [ss[0]]
                for g in range(1, len(ss)):
                    att.append(ss[g])
                    att.append(pv[g - 1])
                att.append(pv[-1])
                ni = 0
                for i, stp in enumerate(att):
                    stp()
                    # next-batch projection finishes ~85% into this window
                    want = min(len(nxt),
                               (i + 1) * 20 * len(nxt) // (17 * len(att)))
                    while ni < want:
                        nxt[ni]()
                        ni += 1
                del state[b]

    nc.compile()
    return nc


def _get_nc():
    global _STATE
    if _STATE is None:
        _STATE = _build()
    return _STATE


def _in_maps(inputs):
    xf = np.ascontiguousarray(
        np.asarray(inputs["hidden_states"], dtype=np.float32).reshape(NSEQ, H))
    maps = []
    for c in range(NCORES):
        sl = slice(c * CSLICE, (c + 1) * CSLICE)
        m = {"x": xf}
        for n, wkey, bkey in (("q", "Wq", "bq"), ("k", "Wk", "bk"),
                              ("v", "Wv", "bv")):
            m[f"w{n}"] = np.ascontiguousarray(
                np.asarray(inputs[wkey], dtype=np.float32)[:, sl])
            m[f"b{n}"] = np.ascontiguousarray(
                np.asarray(inputs[bkey], dtype=np.float32)[sl].reshape(
                    CSLICE, 1))
        maps.append(m)
    return maps


def _assemble(results):
    parts = [results[c]["out"].reshape(B, S, CSLICE) for c in range(NCORES)]
    return np.ascontiguousarray(np.concatenate(parts, axis=-1))


def _run(inputs, trace=False):
    nc = _get_nc()
    maps = _in_maps(inputs)
    last_err = None
    for attempt in range(3):
        try:
            res = run_bass_kernel_spmd(nc, maps,
                                       core_ids=list(range(NCORES)),
                                       trace=trace)
            return _assemble(res.results), res
        except Exception as e:  # transient NRT_EXEC_UNIT_UNRECOVERABLE
            last_err = e
            if attempt < 2:
                import time
                time.sleep(2.0)
    raise last_err


def kernel(**inputs):
    out, _ = _run(inputs, trace=False)
    return out


def run_traced(**inputs):
    out, res = _run(inputs, trace=True)
    return out, res


# revision 63
# speedup vs baseline: 1.0058x; 1.0058x over previous
"""BERT self-attention (no mask) on 8 TRN2 NeuronCores, head-parallel.

Full inputs in, full output out. Core c computes heads 2c and 2c+1 (output
hidden columns [c*128, (c+1)*128)). All matmuls run in bf16 (1 cycle/row at
any width). X^T is produced by DMA xbar transposes of a Pool-engine bf16
copy of X, so the PE array does no transposes at all. Attention scores are
computed transposed (scores^T[k, q]); softmax denominators come from a
ones-column appended to the natural-layout V tiles, and the PV matmul is
oriented with probs as the stationary operand so it streams only 65
columns per key tile. Exp runs on ACT in 1536-wide tiles to amortize the
SBUF access overhead; projection of batch b+1 is interleaved with
attention of batch b.
"""

import numpy as np

try:
    import concourse.bass as bass
except ImportError:  # toolchain not on sys.path in the caller's environment
    import sys
    sys.path.insert(0, "/opt/trn_rl_repo")
    import concourse.bass as bass
import concourse.bacc as bacc
import concourse.mybir as mybir
import concourse.tile as tile
from concourse.bass_utils import run_bass_kernel_spmd

F32 = mybir.dt.float32
BF16 = mybir.dt.bfloat16

B = 4
S = 2048
H = 1024
NH = 16
HD = 64
NSEQ = B * S  # 8192
NCORES = 8
CSLICE = H // NCORES  # 128 hidden cols per core = 2 heads
KCH = H // 128  # 8 contraction tiles for projections
ST = S // 128  # 16 seq tiles of 128 rows per batch
KT = S // 128  # 16 key tiles per (b, head)
QC = 4  # q-chunks of 512 per (b, head)
QW = S // QC  # 512
VW = HD + 1  # V tile width incl. ones column
GW = 3 * 512  # exp group width (3 score blocks)

_STATE = None


def _build():
    nc = bacc.Bacc("TRN2", target_bir_lowering=False, debug=False,
                   num_devices=NCORES)

    x = nc.dram_tensor("x", [NSEQ, H], F32, kind="ExternalInput").ap()
    ws = {n: nc.dram_tensor(f"w{n}", [H, CSLICE], F32, kind="ExternalInput").ap()
          for n in "qkv"}
    bs = {n: nc.dram_tensor(f"b{n}", [CSLICE, 1], F32, kind="ExternalInput").ap()
          for n in "qkv"}
    out = nc.dram_tensor("out", [NSEQ, CSLICE], F32, kind="ExternalOutput").ap()

    with tile.TileContext(nc) as tc:
        with (
            tc.tile_pool(name="persist", bufs=1) as persist,
            tc.tile_pool(name="wstg", bufs=4) as wstg_pool,
            tc.tile_pool(name="xn", bufs=4) as xn_pool,
            tc.tile_pool(name="xnb", bufs=6) as xnb_pool,
            tc.tile_pool(name="xt", bufs=2) as xt_pool,
            tc.tile_pool(name="qkvt", bufs=2) as qkvt_pool,
            tc.tile_pool(name="vp", bufs=4) as vp_pool,
            tc.tile_pool(name="pr", bufs=6) as pr_pool,
            tc.tile_pool(name="rc", bufs=8) as rc_pool,
            tc.tile_pool(name="ot", bufs=4) as ot_pool,
            tc.tile_pool(name="spsum", bufs=2, space="PSUM") as spsum,
            tc.tile_pool(name="cpsum", bufs=1, space="PSUM") as cpsum,
            tc.tile_pool(name="ppsum", bufs=1, space="PSUM") as ppsum,
        ):
            # weights fp32 -> bf16, staged through the ACT DMA queue
            wt = {}
            for n in "qkv":
                for kk in range(KCH):
                    stg = wstg_pool.tile([128, CSLICE], F32,
                                         tag="wstage", name="wstage")
                    nc.scalar.dma_start(stg, ws[n][kk * 128:(kk + 1) * 128, :])
                    t = persist.tile([128, CSLICE], BF16,
                                     tag=f"w{n}{kk}", name=f"w{n}{kk}")
                    nc.vector.tensor_copy(t, stg)
                    wt[n, kk] = t
            bt = {}
            for n in "qkv":
                t = persist.tile([128, 1], F32, tag=f"b{n}", name=f"b{n}")
                nc.scalar.dma_start(t, bs[n])
                bt[n] = t

            def load_cast(b, st0, xnbs):
                # prologue only: SWDGE casting DMA lands bf16 directly,
                # halving descriptor cost and skipping the Pool convert
                for st in range(st0, st0 + 4):
                    r0 = b * S + st * 128
                    xnb = xnb_pool.tile([128, H], BF16, tag="xnb",
                                        name="xnb")
                    nc.gpsimd.dma_start(xnb, x[r0:r0 + 128, :])
                    xnbs[st] = xnb

            def load_quarter(b, st0, xnbs):
                # pair-loads: 2 seq-tiles per DMA halve the issue overhead
                # without a long exclusive hold on the DMA engines
                for h in range(2):
                    r0 = b * S + (st0 + 2 * h) * 128
                    xn = xn_pool.tile([128, 2, H], F32, tag="xn", name="xn")
                    nc.sync.dma_start(
                        xn, x[r0:r0 + 256, :].rearrange(
                            "(j p) c -> p j c", p=128))
                    for j in range(2):
                        xnb = xnb_pool.tile([128, H], BF16, tag="xnb",
                                            name="xnb")
                        nc.gpsimd.tensor_copy(xnb, xn[:, j, :])
                        xnbs[st0 + 2 * h + j] = xnb

            def xt_st(b, st, xnbs, xts):
                # XT[c, st, kk, r] = xnb[r, kk*128 + c]
                nc.sync.dma_start_transpose(xts[:, st, :, :], xnbs.pop(st))

            def proj_mm(n, ci, kk, xts, pps):
                if kk == 0:
                    pps[n, ci] = ppsum.tile([128, QW], F32, tag="pp",
                                            name=f"pp{n}")
                nc.tensor.matmul(
                    pps[n, ci], wt[n, kk], xts[:, 4 * ci:4 * ci + 4, kk, :],
                    start=(kk == 0), stop=(kk == KCH - 1))

            def proj_bias(n, ci, qkvT, pps):
                nc.vector.tensor_scalar_add(
                    qkvT[n][:, ci * QW:(ci + 1) * QW], pps.pop((n, ci)),
                    bt[n])

            def prep_v(b, hl, qkvT, vps):
                # natural-layout V tiles + ones column for the denominator.
                # xbar transpose needs a contiguous destination, so land in
                # tmp and interleave the ones column with a DVE copy.
                vp = vps[hl]
                tmp = xnb_pool.tile([128, KT, HD], BF16, tag="vtmp",
                                    name="vtmp")
                nc.sync.dma_start_transpose(
                    tmp, qkvT["v"][hl * HD:(hl + 1) * HD, :])
                nc.vector.memset(vp[:, :, HD:VW], 1.0)
                nc.vector.tensor_copy(vp[:, :, 0:HD], tmp)

            def score_step(b, blocks, qkvT, prs):
                # scores for a group of blocks (hl, qc, kt), then exp
                gw = len(blocks) * 512
                s_ps = spsum.tile([128, GW], F32, tag="s", name="s")
                for bi, (hl, qc, kt) in enumerate(blocks):
                    p0 = hl * HD
                    nc.tensor.matmul(
                        s_ps[:, bi * 512:(bi + 1) * 512],
                        qkvT["k"][p0:p0 + HD, kt * 128:(kt + 1) * 128],
                        qkvT["q"][p0:p0 + HD, qc * QW:(qc + 1) * QW],
                        start=True, stop=True)
                pr = pr_pool.tile([128, GW], BF16, tag="pr", name="pr")
                nc.scalar.activation(
                    pr[:, 0:gw], s_ps[:, 0:gw],
                    mybir.ActivationFunctionType.Exp,
                    scale=1.0 / np.sqrt(float(HD)))
                prs.append((blocks, pr))

            def pv_step(b, qkvT, vps, ctxs, prs):
                blocks, pr = prs.pop(0)
                for bi, (hl, qc, kt) in enumerate(blocks):
                    if kt == 0:
                        ctxs[hl, qc] = cpsum.tile([128, 4 * VW], F32,
                                                  tag="ctx", name="ctx")
                    ctx = ctxs[hl, qc]
                    # start=True arms the whole PSUM bank (first write per
                    # address replaces, rest accumulate), so arm exactly once
                    # per ctx tile or later slices wipe earlier ones' kt=0.
                    for qs in range(4):
                        nc.tensor.matmul(
                            ctx[:, qs * VW:(qs + 1) * VW],
                            pr[:, bi * 512 + qs * 128:bi * 512 + (qs + 1) * 128],
                            vps[hl][:, kt, :],
                            start=(kt == 0 and qs == 0), stop=(kt == KT - 1),
                            skip_group_check=True)
                    if kt == KT - 1:
                        finish_qc(b, hl, qc, ctxs)

            def finish_qc(b, hl, qc, ctxs):
                ctx = ctxs.pop((hl, qc))
                rc = rc_pool.tile([128, 4], F32, tag="rc", name="rc")
                nc.vector.reciprocal(rc, ctx[:, HD::VW])
                ot = ot_pool.tile([128, 4, HD], F32, tag="ot", name="ot")
                for qs in range(4):
                    nc.vector.tensor_scalar_mul(
                        ot[:, qs, :], ctx[:, qs * VW:qs * VW + HD],
                        rc[:, qs:qs + 1])
                r0 = b * S + qc * QW
                dst = out[r0:r0 + QW, hl * HD:(hl + 1) * HD].rearrange(
                    "(q p) c -> p q c", p=128)
                nc.sync.dma_start(dst, ot)

            def alloc_batch(b):
                xts = xt_pool.tile([128, ST, KCH, 128], BF16,
                                   tag="xt", name="xt")
                qkvT = {n: qkvt_pool.tile([128, S], BF16,
                                          tag=f"{n}T", name=f"{n}T")
                        for n in "qkv"}
                vps = [vp_pool.tile([128, KT, VW], BF16,
                                    tag=f"vp{hl}", name=f"vp{hl}")
                       for hl in range(2)]
                return xts, qkvT, vps

            def proj_steps(b, xts, qkvT, vps, prologue=False):
                # prologue=True interleaves loads with the transposes so the
                # first X^T tiles don't queue behind 24us of load transfers
                # on the DMA engines; in an attention window the loads go
                # up front so they never contend with the previous batch's.
                pps = {}
                xnbs = {}
                steps = []
                if not prologue:
                    for st0 in range(0, ST, 4):
                        steps.append(lambda st0=st0: load_quarter(
                            b, st0, xnbs))
                else:
                    steps.append(lambda: load_cast(b, 0, xnbs))
                for ci in range(QC):
                    if prologue and ci + 1 < QC:
                        steps.append(lambda ci=ci: load_cast(
                            b, 4 * ci + 4, xnbs))
                    steps.append(lambda ci=ci: [
                        xt_st(b, st, xnbs, xts)
                        for st in range(4 * ci, 4 * ci + 4)])
                    for n in "qkv":
                        for kk in range(KCH):
                            steps.append(lambda n=n, ci=ci, kk=kk: proj_mm(
                                n, ci, kk, xts, pps))
                        steps.append(lambda n=n, ci=ci: proj_bias(
                            n, ci, qkvT, pps))
                for hl in range(2):
                    steps.append(lambda hl=hl: prep_v(b, hl, qkvT, vps))
                return steps, None

            def att_lists(b, qkvT, vps):
                # separate score-step and pv-step lists so the driver can
                # run the exp stream ahead of PV (which needs vp ready)
                blocks = [(hl, qc, kt)
                          for hl in range(2) for qc in range(QC)
                          for kt in range(KT)]
                groups = [blocks[i:i + 3] for i in range(0, len(blocks), 3)]
                ctxs = {}
                prs = []
                ss = [lambda g=g: score_step(b, g, qkvT, prs)
                      for g in groups]
                pv = [lambda: pv_step(b, qkvT, vps, ctxs, prs)
                      for _ in groups]
                return ss, pv

            # one continuous scores/PV conveyor across all batches: strict
            # ss(i)/pv(i-1) alternation crosses batch joins so there is no
            # pipeline drain at batch boundaries; projection of batch b+1
            # is paced across batch b's stretch of the stream, finishing
            # ~85% in.
            state = {}
            state[0] = alloc_batch(0)
            for b in range(B):
                ss, pv = att_lists(b, state[b][1], state[b][2])
                if b == 0:
                    for step in proj_steps(0, *state[0])[0]:
                        step()
                nxt = []
                if b + 1 < B:
                    state[b + 1] = alloc_batch(b + 1)
                    nxt, _ = proj_steps(b + 1, *state[b + 1])
                att = [ss[0]]
                for g in range(1, len(ss)):
                    att.append(ss[g])
                    att.append(pv[g - 1])
                att.append(pv[-1])
                ni = 0
                for i, stp in enumerate(att):
                    stp()
                    # next-batch projection finishes ~85% into this window
                    want = min(len(nxt),
                               (i + 1) * 20 * len(nxt) // (17 * len(att)))
                    while ni < want:
                        nxt[ni]()
                        ni += 1
                del state[b]

    nc.compile()
    return nc


def _get_nc():
    global _STATE
    if _STATE is None:
        _STATE = _build()
    return _STATE


def _in_maps(inputs):
    xf = np.ascontiguousarray(
        np.asarray(inputs["hidden_states"], dtype=np.float32).reshape(NSEQ, H))
    maps = []
    for c in range(NCORES):
        sl = slice(c * CSLICE, (c + 1) * CSLICE)
        m = {"x": xf}
        for n, wkey, bkey in (("q", "Wq", "bq"), ("k", "Wk", "bk"),
                              ("v", "Wv", "bv")):
            m[f"w{n}"] = np.ascontiguousarray(
                np.asarray(inputs[wkey], dtype=np.float32)[:, sl])
            m[f"b{n}"] = np.ascontiguousarray(
                np.asarray(inputs[bkey], dtype=np.float32)[sl].reshape(
                    CSLICE, 1))
        maps.append(m)
    return maps


def _assemble(results):
    parts = [results[c]["out"].reshape(B, S, CSLICE) for c in range(NCORES)]
    return np.ascontiguousarray(np.concatenate(parts, axis=-1))


def _run(inputs, trace=False):
    nc = _get_nc()
    maps = _in_maps(inputs)
    last_err = None
    for attempt in range(3):
        try:
            res = run_bass_kernel_spmd(nc, maps,
                                       core_ids=list(range(NCORES)),
                                       trace=trace)
            return _assemble(res.results), res
        except Exception as e:  # transient NRT_EXEC_UNIT_UNRECOVERABLE
            last_err = e
            if attempt < 2:
                import time
                time.sleep(2.0)
    raise last_err


def kernel(**inputs):
    out, _ = _run(inputs, trace=False)
    return out


def run_traced(**inputs):
    out, res = _run(inputs, trace=True)
    return out, res
